# revision 1
# baseline (speedup 1.0000x reference)
"""Trainium2 Bass kernel for nn_Conv2d_20590073217670.

Conv2d: input [32,64,64,64] (NCHW), weight [576,128] (unfold layout:
row = ci*9 + a*3 + b for tap (a,b)), bias [1,128,1,1], stride 1, pad 1.
Output [32,128,64,64].

Strategy: data-parallel over batch — 4 images per NeuronCore, 8 cores.
Per image, implicit GEMM: out[co, y, x] = sum_{a,b,ci} W[ci,a,b,co] *
img[ci, y+a-1, x+b-1].  SBUF holds the image on partitions 0:64 and a
one-row-down shifted copy on partitions 64:128, so a single K=128
matmul accumulates two vertical taps (a, a+1) at once.  The rounded
fp32r image is stored column-padded ([128, 64, 66], zero borders), so
every matmul is a full 64-wide slide satisfying the fp32r ISA
restrictions (even innermost count, 8B-aligned full-bank PSUM output).
Row borders are handled by restricting output rows (PSUM has_written
zero-fill keeps partial accumulation exact).  DVE produces all matmul
inputs (fp32->fp32r rounding) and evicts PSUM with a fused bias add.
"""
import sys

for _p in ("/opt/trn_rl_repo", "/root/.axon_site/_ro/trn_rl_repo"):
    if _p not in sys.path:
        sys.path.append(_p)

import numpy as np
from contextlib import ExitStack

import concourse.bacc as bacc
import concourse.tile as tile
from concourse import mybir
from concourse.bass_utils import run_bass_kernel_spmd

f32 = mybir.dt.float32
f32r = mybir.dt.float32r

N_CORES = 8
NB = 4  # images per core


def build_nc():
    nc = bacc.Bacc()
    x = nc.declare_dram_parameter("x", [NB, 64, 64, 64], f32, isOutput=False)
    w = nc.declare_dram_parameter("w", [576, 128], f32, isOutput=False)
    bias = nc.declare_dram_parameter("b", [128, 1], f32, isOutput=False)
    out = nc.declare_dram_parameter("out", [NB, 128, 64, 64], f32, isOutput=True)

    with tile.TileContext(nc) as tc, ExitStack() as ctx:
        const = ctx.enter_context(tc.tile_pool(name="const", bufs=1))
        xs_pool = ctx.enter_context(tc.tile_pool(name="xs", bufs=3))
        xr_pool = ctx.enter_context(tc.tile_pool(name="xr", bufs=3))
        ob_pool = ctx.enter_context(tc.tile_pool(name="ob", bufs=2))
        ps_pool = ctx.enter_context(tc.tile_pool(name="ps", bufs=8, space="PSUM"))

        # ---- weights: one [128, 9, 128] tile; partition p<64 holds channel
        # p's taps 0..8, partition 64+ci holds channel ci's taps 3..8 at
        # slots 0..5 (tap axis pre-shifted by -3).  Then the lhsT view
        # wr[:, t, :] pairs taps (t, t+3) across the partition halves:
        #   t in 0..2  -> taps (0,b) & (1,b)
        #   t in 3..5  -> taps (1,b) & (2,b)
        w3 = w[:].rearrange("(c t) m -> c t m", t=9)
        ws = const.tile([128, 9, 128], f32)
        wr = const.tile([128, 9, 128], f32r)
        bt = const.tile([128, 1], f32)
        zc = const.tile([128, 64, 1], f32)
        nc.sync.dma_start(out=ws[0:64, :, :], in_=w3)
        nc.sync.dma_start(out=ws[64:128, 0:6, :], in_=w3[:, 3:9, :])
        nc.sync.dma_start(out=bt[:], in_=bias[:])
        nc.vector.memset(zc[:], 0.0)
        nc.vector.tensor_copy(wr[0:64, :, :], ws[0:64, :, :])
        nc.vector.tensor_copy(wr[64:128, 0:6, :], ws[64:128, 0:6, :])

        for n in range(NB):
            xs = xs_pool.tile([128, 64, 64], f32)
            xr = xr_pool.tile([128, 64, 66], f32r)
            # image rows on partitions 0:64; one-row-down copy on 64:128
            nc.sync.dma_start(out=xs[0:64, :, :], in_=x[n])
            nc.sync.dma_start(out=xs[64:128, 0:63, :], in_=xs[0:64, 1:64, :])
            # fp32 -> fp32r rounding (DVE) into the column-padded layout;
            # upper-half row 63 is never read.  Zero border columns.
            nc.vector.tensor_copy(xr[:, 0:63, 1:65], xs[:, 0:63, :])
            nc.vector.tensor_copy(xr[0:64, 63, 1:65], xs[0:64, 63, :])
            nc.vector.tensor_copy(xr[:, :, 0:1], zc[:])
            nc.vector.tensor_copy(xr[:, :, 65:66], zc[:])

            osb = ob_pool.tile([128, 64, 64], f32)
            for blk in range(8):
                y0 = blk * 8
                P = ps_pool.tile([128, 8, 64], f32)
                if blk == 0:
                    pair_t, pr0 = 3, 0      # taps (1,2), rhs rows y0..y0+7
                else:
                    pair_t, pr0 = 0, y0 - 1  # taps (0,1), rhs rows y0-1..y0+6
                # b=1 first: full [8,64] coverage zero-fills the whole bank
                for k, b in enumerate((1, 0, 2)):
                    nc.tensor.matmul(
                        P[:, 0:8, :],
                        wr[:, pair_t + b, :],
                        xr[:, pr0:pr0 + 8, b:b + 64],
                        start=(k == 0), stop=False,
                    )
                # remaining vertical tap as K=64 single on partitions 0:64
                for k, b in enumerate((1, 0, 2)):
                    last = k == 2
                    if blk == 0:
                        # tap (0,b): out rows 1..7 read img rows 0..6
                        nc.tensor.matmul(
                            P[:, 1:8, :], wr[0:64, b, :],
                            xr[0:64, 0:7, b:b + 64],
                            start=False, stop=last,
                        )
                    elif blk == 7:
                        # tap (2,b): out rows 56..62 read img rows 57..63
                        nc.tensor.matmul(
                            P[:, 0:7, :], wr[0:64, 6 + b, :],
                            xr[0:64, 57:64, b:b + 64],
                            start=False, stop=last,
                        )
                    else:
                        nc.tensor.matmul(
                            P[:, 0:8, :], wr[0:64, 6 + b, :],
                            xr[0:64, y0 + 1:y0 + 9, b:b + 64],
                            start=False, stop=last,
                        )
                nc.vector.tensor_scalar_add(osb[:, y0:y0 + 8, :], P[:, :, :], bt[:])

            nc.sync.dma_start(out=out[n], in_=osb[:])

    nc.finalize()
    return nc


_NC = None


def _get_nc():
    global _NC
    if _NC is None:
        _NC = build_nc()
    return _NC


def kernel(**inputs) -> np.ndarray:
    x = np.ascontiguousarray(np.asarray(inputs["input"], dtype=np.float32))
    w = np.ascontiguousarray(np.asarray(inputs["weight"], dtype=np.float32))
    b = np.ascontiguousarray(
        np.asarray(inputs["bias"], dtype=np.float32).reshape(128, 1))
    nc = _get_nc()
    in_maps = [
        {"x": x[c * NB:(c + 1) * NB], "w": w, "b": b} for c in range(N_CORES)
    ]
    res = run_bass_kernel_spmd(nc, in_maps, list(range(N_CORES)))
    return np.concatenate([r["out"] for r in res.results], axis=0)



# revision 3
# speedup vs baseline: 1.0452x; 1.0452x over previous
"""Trainium2 Bass kernel for nn_Conv2d_20590073217670.

Conv2d: input [32,64,64,64] (NCHW), weight [576,128] (unfold layout:
row = ci*9 + a*3 + b for tap (a,b)), bias [1,128,1,1], stride 1, pad 1.
Output [32,128,64,64].

Strategy: data-parallel over batch - 4 images per NeuronCore, 8 cores.
All matmuls run in bf16 (4x the fp32r PE rate); the rel-err budget
(2e-2) dwarfs bf16 rounding (~5e-3 measured).  Host converts inputs to
bf16 and upcasts the bf16 output back to fp32.

Per image, implicit GEMM over the 9 taps with K=128 tap-pairing.  Two
SBUF layouts of the image provide the two pair geometries:
  A2: partitions 0:64  = img[r, c]   (column-pad layout, img col c at
      tile col c+1), partitions 64:128 = img[r, c+1] (img col c at
      tile col c).  One K=128 matmul on A2 accumulates taps
      (a,0)+(a,1) for any vertical tap a via the AP row offset.
  A:  partitions 0:64 = img[r, c], partitions 64:128 = img[r+1, c]
      (both at tile col c+1).  One K=128 matmul accumulates the
      vertical pair (0,2)+(1,2) at column offset +1.
Per 8-row output block that gives 5 matmuls instead of 9:
  3x col-pairs (a=0,1,2) on A2, 1x row-pair (0,2)+(1,2) on A, and a
  K=64 single for tap (2,2) on A's lower half.  Borders are handled by
  restricting output rows (PSUM has_written keeps partial sums exact)
  and zeroed pad columns.  ScalarE evicts PSUM with a fused bias add
  (bf16 out); DVE only zeroes pad columns.
"""
import sys

for _p in ("/opt/trn_rl_repo", "/root/.axon_site/_ro/trn_rl_repo"):
    if _p not in sys.path:
        sys.path.append(_p)

import numpy as np
import ml_dtypes
from contextlib import ExitStack

import concourse.bacc as bacc
import concourse.tile as tile
from concourse import mybir
from concourse.bass_utils import run_bass_kernel_spmd

f32 = mybir.dt.float32
bf16 = mybir.dt.bfloat16

N_CORES = 8
NB = 4  # images per core


def build_nc():
    nc = bacc.Bacc()
    x = nc.declare_dram_parameter("x", [NB, 64, 64, 64], bf16, isOutput=False)
    w = nc.declare_dram_parameter("w", [576, 128], bf16, isOutput=False)
    bias = nc.declare_dram_parameter("b", [128, 1], f32, isOutput=False)
    out = nc.declare_dram_parameter("out", [NB, 128, 64, 64], bf16, isOutput=True)

    with tile.TileContext(nc) as tc, ExitStack() as ctx:
        const = ctx.enter_context(tc.tile_pool(name="const", bufs=1))
        a2_pool = ctx.enter_context(tc.tile_pool(name="a2", bufs=2))
        a_pool = ctx.enter_context(tc.tile_pool(name="a", bufs=2))
        ob_pool = ctx.enter_context(tc.tile_pool(name="ob", bufs=2))
        ps_pool = ctx.enter_context(tc.tile_pool(name="ps", bufs=8, space="PSUM"))

        # ---- weights: [128, 5, 128]; slot s pairs tap u (parts 0:64) with
        # tap l (parts 64:128), taps indexed t = 3a + b:
        #   slot 0: (0,0)+(0,1)   slot 1: (1,0)+(1,1)   slot 2: (2,0)+(2,1)
        #   slot 3: (0,2)+(1,2)   slot 4: (1,2)+(2,2)
        w3 = w[:].rearrange("(c t) m -> c t m", t=9)
        WT = const.tile([128, 5, 128], bf16)
        bt = const.tile([128, 1], f32)
        for s, (u, l) in enumerate(((0, 1), (3, 4), (6, 7), (2, 5), (5, 8))):
            nc.sync.dma_start(out=WT[0:64, s, :], in_=w3[:, u, :])
            nc.sync.dma_start(out=WT[64:128, s, :], in_=w3[:, l, :])
        nc.sync.dma_start(out=bt[:], in_=bias[:])

        act_id = mybir.ActivationFunctionType.Identity

        for n in range(NB):
            A2 = a2_pool.tile([128, 64, 66], bf16)
            A = a_pool.tile([128, 64, 66], bf16)
            # pad columns (never-read cols are left untouched)
            nc.vector.memset(A2[0:64, :, 0:1], 0.0)
            nc.vector.memset(A2[0:64, :, 65:66], 0.0)
            nc.vector.memset(A2[64:128, :, 64:66], 0.0)
            nc.vector.memset(A[64:128, :, 65:66], 0.0)
            # A2 upper: img col c at tile col c+1; lower: img col c at c
            nc.sync.dma_start(out=A2[0:64, :, 1:65], in_=x[n])
            nc.sync.dma_start(out=A2[64:128, :, 0:64], in_=A2[0:64, :, 1:65])
            # A upper: copy of A2 upper (borders included); lower: one row up
            nc.sync.dma_start(out=A[0:64, :, :], in_=A2[0:64, :, :])
            nc.sync.dma_start(out=A[64:128, 0:63, 1:65], in_=A2[0:64, 1:64, 1:65])

            osb = ob_pool.tile([128, 64, 64], bf16)
            for blk in range(8):
                y0 = blk * 8
                P = ps_pool.tile([128, 8, 64], f32)
                first = True

                def mm(out_ap, wt_ap, rhs_ap, stop=False):
                    nonlocal first
                    nc.tensor.matmul(out_ap, wt_ap, rhs_ap,
                                     start=first, stop=stop)
                    first = False

                # 3 column-pairs (a,0)+(a,1), a = 0,1,2: rhs rows y0-1+a.
                # Full-coverage pair first: the start=True matmul must cover
                # the whole PSUM region (blk 0's a=0 pair is row-restricted).
                for a in ((1, 2, 0) if blk == 0 else (0, 1, 2)):
                    r0 = y0 - 1 + a
                    if r0 < 0:      # blk 0, a=0: skip output row 0
                        mm(P[:, 1:8, :], WT[:, a, :], A2[:, 0:7, 0:64])
                    elif r0 + 8 > 64:  # blk 7, a=2: skip output row 63
                        mm(P[:, 0:7, :], WT[:, a, :], A2[:, 57:64, 0:64])
                    else:
                        mm(P[:, 0:8, :], WT[:, a, :], A2[:, r0:r0 + 8, 0:64])

                if blk == 0:
                    # row-pair (1,2)+(2,2) covers all 8 rows; single (0,2)
                    # covers rows 1..7
                    mm(P[:, 0:8, :], WT[:, 4, :], A[:, 0:8, 2:66])
                    mm(P[:, 1:8, :], WT[0:64, 3, :], A2[0:64, 0:7, 2:66],
                       stop=True)
                else:
                    # row-pair (0,2)+(1,2); single (2,2) on A lower half
                    mm(P[:, 0:8, :], WT[:, 3, :], A[:, y0 - 1:y0 + 7, 2:66])
                    if blk == 7:
                        mm(P[:, 0:7, :], WT[64:128, 4, :],
                           A[64:128, 56:63, 2:66], stop=True)
                    else:
                        mm(P[:, 0:8, :], WT[64:128, 4, :],
                           A[64:128, y0:y0 + 8, 2:66], stop=True)

                nc.scalar.activation(osb[:, y0:y0 + 8, :], P[:, :, :],
                                     act_id, bias=bt[:])

            nc.sync.dma_start(out=out[n], in_=osb[:])

    nc.finalize()
    return nc


_NC = None


def _get_nc():
    global _NC
    if _NC is None:
        _NC = build_nc()
    return _NC


def kernel(**inputs) -> np.ndarray:
    x = np.ascontiguousarray(
        np.asarray(inputs["input"], dtype=np.float32)).astype(ml_dtypes.bfloat16)
    w = np.ascontiguousarray(
        np.asarray(inputs["weight"], dtype=np.float32)).astype(ml_dtypes.bfloat16)
    b = np.ascontiguousarray(
        np.asarray(inputs["bias"], dtype=np.float32).reshape(128, 1))
    nc = _get_nc()
    in_maps = [
        {"x": x[c * NB:(c + 1) * NB], "w": w, "b": b} for c in range(N_CORES)
    ]
    res = run_bass_kernel_spmd(nc, in_maps, list(range(N_CORES)))
    full = np.concatenate([r["out"] for r in res.results], axis=0)
    return full.astype(np.float32)


# revision 4
# speedup vs baseline: 1.7295x; 1.6547x over previous
"""Trainium2 Bass kernel for nn_Conv2d_20590073217670.

Conv2d: input [32,64,64,64] (NCHW), weight [576,128] (unfold layout:
row = ci*9 + a*3 + b for tap (a,b)), bias [1,128,1,1], stride 1, pad 1.
Output [32,128,64,64].

Strategy: data-parallel over batch - 4 images per NeuronCore, 8 cores.
All matmuls run in bf16 (4x the fp32r PE rate); the rel-err budget
(2e-2) dwarfs bf16 rounding (~3e-3 measured).  Host converts inputs to
bf16 and upcasts the bf16 output back to fp32.

Implicit GEMM over the 9 taps with K=128 tap-pairing.  Tiles are DENSE
[128, 64, 64] (8 KiB/partition) so every DMA is a single contiguous
chunk per partition; the +-1-column / +-1-row tap shifts are realized
as 1- and 63-element shifts of the flattened per-partition image
vector, with the wrap-garbage columns zeroed by tiny DVE memsets
(which exactly reproduces the zero-pad border semantics):
  Tc: parts 0:64 = img[r,c], parts 64:128 = img[r,c+1] (col 63 -> 0)
  Td: parts 0:64 = img[r,c-1] (col 0 -> 0),
      parts 64:128 = img[r+1,c-1] (col 0 -> 0)
Per 8-row output block, 5 full-width matmuls accumulate one PSUM bank:
  (a,1)+(a,2) pairs on Tc for a=0,1,2; (0,0)+(1,0) pair on Td; and a
  K=64 single (2,0) on Td's lower half.  Vertical borders restrict
  output rows (PSUM has_written keeps partial sums exact; the first,
  start=True matmul always covers the full bank).  ScalarE evicts
  PSUM with a fused bias add to bf16; all PSUM reads/writes are
  contiguous.
"""
import sys

for _p in ("/opt/trn_rl_repo", "/root/.axon_site/_ro/trn_rl_repo"):
    if _p not in sys.path:
        sys.path.append(_p)

import numpy as np
import ml_dtypes
from contextlib import ExitStack

import concourse.bacc as bacc
import concourse.tile as tile
from concourse import mybir
from concourse.bass_utils import run_bass_kernel_spmd

f32 = mybir.dt.float32
bf16 = mybir.dt.bfloat16

N_CORES = 8
NB = 4  # images per core


def build_nc():
    nc = bacc.Bacc()
    x = nc.declare_dram_parameter("x", [NB, 64, 64, 64], bf16, isOutput=False)
    w = nc.declare_dram_parameter("w", [576, 128], bf16, isOutput=False)
    bias = nc.declare_dram_parameter("b", [128, 1], f32, isOutput=False)
    out = nc.declare_dram_parameter("out", [NB, 128, 64, 64], bf16, isOutput=True)

    with tile.TileContext(nc) as tc, ExitStack() as ctx:
        const = ctx.enter_context(tc.tile_pool(name="const", bufs=1))
        tc_pool = ctx.enter_context(tc.tile_pool(name="tc", bufs=2))
        td_pool = ctx.enter_context(tc.tile_pool(name="td", bufs=2))
        ob_pool = ctx.enter_context(tc.tile_pool(name="ob", bufs=2))
        ps_pool = ctx.enter_context(tc.tile_pool(name="ps", bufs=8, space="PSUM"))

        # ---- weights: [128, 5, 128]; slot s pairs tap u (parts 0:64) with
        # tap l (parts 64:128), taps indexed t = 3a + b:
        #   slot 0: (0,1)+(0,2)   slot 1: (1,1)+(1,2)   slot 2: (2,1)+(2,2)
        #   slot 3: (0,0)+(1,0)   slot 4: (1,0)+(2,0)
        w3 = w[:].rearrange("(c t) m -> c t m", t=9)
        WT = const.tile([128, 5, 128], bf16)
        bt = const.tile([128, 1], f32)
        for s, (u, l) in enumerate(((1, 2), (4, 5), (7, 8), (0, 3), (3, 6))):
            nc.sync.dma_start(out=WT[0:64, s, :], in_=w3[:, u, :])
            nc.sync.dma_start(out=WT[64:128, s, :], in_=w3[:, l, :])
        nc.sync.dma_start(out=bt[:], in_=bias[:])

        act_id = mybir.ActivationFunctionType.Identity

        for n in range(NB):
            Tc = tc_pool.tile([128, 64, 64], bf16)
            Td = td_pool.tile([128, 64, 64], bf16)
            Tcf = Tc[:].rearrange("p r c -> p (r c)")
            Tdf = Td[:].rearrange("p r c -> p (r c)")
            # img into Tc upper; all shifts are contiguous flat copies
            nc.sync.dma_start(out=Tc[0:64, :, :], in_=x[n])
            nc.sync.dma_start(out=Tcf[64:128, 0:4095], in_=Tcf[0:64, 1:4096])
            nc.sync.dma_start(out=Tdf[0:64, 1:4096], in_=Tcf[0:64, 0:4095])
            nc.sync.dma_start(out=Tdf[64:128, 0:4033], in_=Tcf[0:64, 63:4096])
            # zero the wrap columns (= conv zero-pad border semantics)
            nc.vector.memset(Tc[64:128, :, 63:64], 0.0)
            nc.vector.memset(Td[0:64, :, 0:1], 0.0)
            nc.vector.memset(Td[64:128, :, 0:1], 0.0)

            osb = ob_pool.tile([128, 64, 64], bf16)
            for blk in range(8):
                y0 = blk * 8
                P = ps_pool.tile([128, 8, 64], f32)
                first = True

                def mm(out_ap, wt_ap, rhs_ap, stop=False):
                    nonlocal first
                    nc.tensor.matmul(out_ap, wt_ap, rhs_ap,
                                     start=first, stop=stop)
                    first = False

                if blk == 0:
                    mm(P[:, 0:8, :], WT[:, 1, :], Tc[:, 0:8, :])
                    mm(P[:, 0:8, :], WT[:, 2, :], Tc[:, 1:9, :])
                    mm(P[:, 1:8, :], WT[:, 0, :], Tc[:, 0:7, :])
                    mm(P[:, 0:8, :], WT[:, 4, :], Td[:, 0:8, :])
                    mm(P[:, 1:8, :], WT[0:64, 3, :], Td[0:64, 0:7, :],
                       stop=True)
                elif blk == 7:
                    mm(P[:, 0:8, :], WT[:, 1, :], Tc[:, 56:64, :])
                    mm(P[:, 0:8, :], WT[:, 0, :], Tc[:, 55:63, :])
                    mm(P[:, 0:7, :], WT[:, 2, :], Tc[:, 57:64, :])
                    mm(P[:, 0:8, :], WT[:, 3, :], Td[:, 55:63, :])
                    mm(P[:, 0:7, :], WT[64:128, 4, :], Td[64:128, 56:63, :],
                       stop=True)
                else:
                    mm(P[:, 0:8, :], WT[:, 1, :], Tc[:, y0:y0 + 8, :])
                    mm(P[:, 0:8, :], WT[:, 0, :], Tc[:, y0 - 1:y0 + 7, :])
                    mm(P[:, 0:8, :], WT[:, 2, :], Tc[:, y0 + 1:y0 + 9, :])
                    mm(P[:, 0:8, :], WT[:, 3, :], Td[:, y0 - 1:y0 + 7, :])
                    mm(P[:, 0:8, :], WT[64:128, 4, :], Td[64:128, y0:y0 + 8, :],
                       stop=True)

                nc.scalar.activation(osb[:, y0:y0 + 8, :], P[:, :, :],
                                     act_id, bias=bt[:])

            nc.sync.dma_start(out=out[n], in_=osb[:])

    nc.finalize()
    return nc


_NC = None


def _get_nc():
    global _NC
    if _NC is None:
        _NC = build_nc()
    return _NC


def kernel(**inputs) -> np.ndarray:
    x = np.ascontiguousarray(
        np.asarray(inputs["input"], dtype=np.float32)).astype(ml_dtypes.bfloat16)
    w = np.ascontiguousarray(
        np.asarray(inputs["weight"], dtype=np.float32)).astype(ml_dtypes.bfloat16)
    b = np.ascontiguousarray(
        np.asarray(inputs["bias"], dtype=np.float32).reshape(128, 1))
    nc = _get_nc()
    in_maps = [
        {"x": x[c * NB:(c + 1) * NB], "w": w, "b": b} for c in range(N_CORES)
    ]
    res = run_bass_kernel_spmd(nc, in_maps, list(range(N_CORES)))
    full = np.concatenate([r["out"] for r in res.results], axis=0)
    return full.astype(np.float32)


# revision 5
# speedup vs baseline: 2.3249x; 1.3442x over previous
"""Trainium2 Bass kernel for nn_Conv2d_20590073217670.

Conv2d: input [32,64,64,64] (NCHW), weight [576,128] (unfold layout:
row = ci*9 + a*3 + b for tap (a,b)), bias [1,128,1,1], stride 1, pad 1.
Output [32,128,64,64].

Strategy: data-parallel over batch - 4 images per NeuronCore, 8 cores.
All matmuls run in bf16 (4x the fp32r PE rate); the rel-err budget
(2e-2) dwarfs bf16 rounding (~3e-3 measured).  Host converts inputs to
bf16 and upcasts the bf16 output back to fp32.

Implicit GEMM over the 9 taps with K=128 tap-pairing.  Tiles are DENSE
[128, 64, 64] (8 KiB/partition) so every DMA is a single contiguous
chunk per partition; the +-1-column / +-1-row tap shifts are realized
as 1- and 63-element shifts of the flattened per-partition image
vector, sourced straight from HBM (partition-shifted halves) or DVE
(same-partition shift), with wrap-garbage columns zeroed by tiny
memsets (= the conv zero-pad border semantics):
  Tc: parts 0:64 = img[r,c], parts 64:128 = img[r,c+1] (col 63 -> 0)
  Td: parts 0:64 = img[r,c-1] (col 0 -> 0),
      parts 64:128 = img[r+1,c-1] (col 0 -> 0)
Per 8-row output block, 5 full-width matmuls accumulate one PSUM bank:
  (a,1)+(a,2) pairs on Tc for a=0,1,2; (0,0)+(1,0) pair on Td; and a
  K=64 single (2,0) on Td's lower half.  The matmul sweep runs
  weight-slot-major over 4-block half-images so consecutive matmuls
  share the stationary operand (amortizes LDWEIGHTS).  Vertical
  borders restrict output rows (PSUM has_written keeps partial sums
  exact; each bank's first matmul covers it fully).  ScalarE evicts
  4-bank PSUM tiles with a fused bias add to bf16.
"""
import sys

for _p in ("/opt/trn_rl_repo", "/root/.axon_site/_ro/trn_rl_repo"):
    if _p not in sys.path:
        sys.path.append(_p)

import numpy as np
import ml_dtypes
from contextlib import ExitStack

import concourse.bacc as bacc
import concourse.tile as tile
from concourse import mybir
from concourse.bass_utils import run_bass_kernel_spmd

f32 = mybir.dt.float32
bf16 = mybir.dt.bfloat16

N_CORES = 8
NB = 4  # images per core


def build_nc():
    nc = bacc.Bacc()
    x = nc.declare_dram_parameter("x", [NB, 64, 64, 64], bf16, isOutput=False)
    w = nc.declare_dram_parameter("w", [576, 128], bf16, isOutput=False)
    bias = nc.declare_dram_parameter("b", [128, 1], f32, isOutput=False)
    out = nc.declare_dram_parameter("out", [NB, 128, 64, 64], bf16, isOutput=True)

    with tile.TileContext(nc) as tc, ExitStack() as ctx:
        const = ctx.enter_context(tc.tile_pool(name="const", bufs=1))
        tc_pool = ctx.enter_context(tc.tile_pool(name="tc", bufs=2))
        td_pool = ctx.enter_context(tc.tile_pool(name="td", bufs=2))
        ob_pool = ctx.enter_context(tc.tile_pool(name="ob", bufs=4))
        ps_pool = ctx.enter_context(tc.tile_pool(name="ps", bufs=2, space="PSUM"))

        # ---- weights: [128, 5, 128]; slot s pairs tap u (parts 0:64) with
        # tap l (parts 64:128), taps indexed t = 3a + b:
        #   slot 0: (0,1)+(0,2)   slot 1: (1,1)+(1,2)   slot 2: (2,1)+(2,2)
        #   slot 3: (0,0)+(1,0)   slot 4: (1,0)+(2,0)
        w3 = w[:].rearrange("(c t) m -> c t m", t=9)
        WT = const.tile([128, 5, 128], bf16)
        bt = const.tile([128, 1], f32)
        for s, (u, l) in enumerate(((1, 2), (4, 5), (7, 8), (0, 3), (3, 6))):
            nc.sync.dma_start(out=WT[0:64, s, :], in_=w3[:, u, :])
            nc.sync.dma_start(out=WT[64:128, s, :], in_=w3[:, l, :])
        nc.sync.dma_start(out=bt[:], in_=bias[:])

        act_id = mybir.ActivationFunctionType.Identity

        for n in range(NB):
            Tc = tc_pool.tile([128, 64, 64], bf16)
            Td = td_pool.tile([128, 64, 64], bf16)
            Tcf = Tc[:].rearrange("p r c -> p (r c)")
            Tdf = Td[:].rearrange("p r c -> p (r c)")
            xf = x[n].rearrange("c r w -> c (r w)")
            # contiguous flat loads; partition-shifted halves come straight
            # from HBM, the same-partition shift (Td upper) goes via DVE
            nc.sync.dma_start(out=Tcf[0:64, :], in_=xf[:, :])
            nc.sync.dma_start(out=Tcf[64:128, 0:4095], in_=xf[:, 1:4096])
            nc.sync.dma_start(out=Tdf[64:128, 0:4033], in_=xf[:, 63:4096])
            nc.vector.tensor_copy(Tdf[0:64, 1:4096], Tcf[0:64, 0:4095])
            # zero the wrap columns (= conv zero-pad border semantics)
            nc.vector.memset(Tc[64:128, :, 63:64], 0.0)
            nc.vector.memset(Td[0:64, :, 0:1], 0.0)
            nc.vector.memset(Td[64:128, :, 0:1], 0.0)

            for half in range(2):
                blks = range(half * 4, half * 4 + 4)
                r0 = half * 32
                P = ps_pool.tile([128, 32, 64], f32)  # 4 PSUM banks
                osb = ob_pool.tile([128, 32, 64], bf16)

                def pr(blk, lo=0, hi=8):
                    q0 = (blk % 4) * 8
                    return P[:, q0 + lo:q0 + hi, :]

                # slot-major sweep; slot 1 first: full coverage on every bank
                for blk in blks:
                    y0 = blk * 8
                    nc.tensor.matmul(pr(blk), WT[:, 1, :], Tc[:, y0:y0 + 8, :],
                                     start=True, stop=False)
                for blk in blks:
                    y0 = blk * 8
                    if blk == 0:
                        nc.tensor.matmul(pr(blk, 1, 8), WT[:, 0, :],
                                         Tc[:, 0:7, :], start=False, stop=False)
                    else:
                        nc.tensor.matmul(pr(blk), WT[:, 0, :],
                                         Tc[:, y0 - 1:y0 + 7, :],
                                         start=False, stop=False)
                for blk in blks:
                    y0 = blk * 8
                    if blk == 7:
                        nc.tensor.matmul(pr(blk, 0, 7), WT[:, 2, :],
                                         Tc[:, 57:64, :], start=False, stop=False)
                    else:
                        nc.tensor.matmul(pr(blk), WT[:, 2, :],
                                         Tc[:, y0 + 1:y0 + 9, :],
                                         start=False, stop=False)
                # slot 3: dp01 pairs (blk >= 1), then blk 0's (0,0) single
                for blk in blks:
                    y0 = blk * 8
                    if blk != 0:
                        nc.tensor.matmul(pr(blk), WT[:, 3, :],
                                         Td[:, y0 - 1:y0 + 7, :],
                                         start=False, stop=False)
                if half == 0:
                    nc.tensor.matmul(pr(0, 1, 8), WT[0:64, 3, :],
                                     Td[0:64, 0:7, :], start=False, stop=False)
                    # slot 4: blk 0's dp12 pair (its last), then (2,0) singles
                    nc.tensor.matmul(pr(0), WT[:, 4, :], Td[:, 0:8, :],
                                     start=False, stop=True)
                for blk in blks:
                    y0 = blk * 8
                    if blk == 0:
                        continue
                    if blk == 7:
                        nc.tensor.matmul(pr(blk, 0, 7), WT[64:128, 4, :],
                                         Td[64:128, 56:63, :],
                                         start=False, stop=True)
                    else:
                        nc.tensor.matmul(pr(blk), WT[64:128, 4, :],
                                         Td[64:128, y0:y0 + 8, :],
                                         start=False, stop=True)

                nc.scalar.activation(osb[:], P[:], act_id, bias=bt[:])
                nc.sync.dma_start(out=out[n][:, r0:r0 + 32, :], in_=osb[:])

    nc.finalize()
    return nc


_NC = None


def _get_nc():
    global _NC
    if _NC is None:
        _NC = build_nc()
    return _NC


def kernel(**inputs) -> np.ndarray:
    x = np.ascontiguousarray(
        np.asarray(inputs["input"], dtype=np.float32)).astype(ml_dtypes.bfloat16)
    w = np.ascontiguousarray(
        np.asarray(inputs["weight"], dtype=np.float32)).astype(ml_dtypes.bfloat16)
    b = np.ascontiguousarray(
        np.asarray(inputs["bias"], dtype=np.float32).reshape(128, 1))
    nc = _get_nc()
    in_maps = [
        {"x": x[c * NB:(c + 1) * NB], "w": w, "b": b} for c in range(N_CORES)
    ]
    res = run_bass_kernel_spmd(nc, in_maps, list(range(N_CORES)))
    full = np.concatenate([r["out"] for r in res.results], axis=0)
    return full.astype(np.float32)
